# revision 58
# baseline (speedup 1.0000x reference)
"""GCN (3-layer, PyG GCNConv-style) forward pass on 8 Trainium2 NeuronCores.

Strategy (dst-sharded graph parallelism, v3):
  - Nodes partitioned contiguously across 8 cores (2560/core, 20 tiles of
    128 dst slots), degree-balanced by a snake deal over buckets.
  - GCN normalization is folded into the host-built selector matrices:
    sel[lane, slot] = dis[src]*dis[dst] for edge lane->slot (dis=deg^-1/2,
    self-loops included).  A per-tile bias lane (sel=1.0) gathers a bias row
    appended to the half-b AllGather buffer, so out = sum(sel*Z[src]) + b
    comes straight out of PSUM with no vector post-ops.
  - Per layer: Z = H @ Wl (unscaled, bf16) -> AllGather in two src halves
    (a = first 7 tiles of every core's shard, b = rest + bias row).
  - Edge-source rows are gathered with SWDGE dma_gather in PREPARE_ONLY
    mode, split across all 4 SWDGE queues (4 Q7 DSP pairs generate
    descriptors concurrently, ahead of data availability); trigger_dma
    fires each queue's descriptors the moment its AllGather half lands.
  - Aggregation is matmul with the selector.  Layers 1-2 run "transposed"
    (lhsT = gathered Z chunk, rhs = selector) so the accumulated PSUM is
    H^T directly: relu on the scalar engine yields the lhsT for the next
    layer's Z matmul with no PE transposes.  Layer 3 runs normal
    orientation so the output lands as [slot, feat].
  - Cross-phase accumulation: half-a partials spill to bf16 SBUF via the
    scalar engine and are re-injected into the half-b PSUM group with an
    identity matmul.
"""

import os
import sys

import numpy as np

sys.path.insert(0, "/opt/trn_rl_repo")

import ml_dtypes  # noqa: E402

import concourse.bass as bass  # noqa: E402
import concourse.bacc as bacc  # noqa: E402
import concourse.mybir as mybir  # noqa: E402
from concourse.bass_utils import run_bass_kernel_spmd  # noqa: E402
from concourse.library_config import mlp as _mlp_lib  # noqa: E402
from concourse.tile import TileContext  # noqa: E402
from concourse.tile_rust import add_dep_helper  # noqa: E402

BF16 = ml_dtypes.bfloat16

# ----------------------------------------------------------------------------
# Problem configuration (hardcoded for nn_Encoder_17386027614431)
# ----------------------------------------------------------------------------
N_NODES = 20000
N_CORES = 8
T = 128          # dst slots per tile (= SBUF partitions)
NT = 20          # tiles per core
SHARD = NT * T   # 2560 node slots per core
D0 = 256                 # input feature dim
DL = [256, 128, 128]     # per-layer aggregated dims (layer 3 padded 64->128)
D3_REAL = 64
NQ = 4                   # SWDGE queues (Q7 DSP pairs)
HT_A = 7                 # z tiles in src half a
HALF_A = HT_A * T        # 896
NB1 = NT - HT_A          # 13 z tiles in half b
HB_ROWS = NB1 * T + T    # half-b rows: z rows + bias/pad tile
NCH_A = 8                # gather chunks in phase A
NCH_B = 10               # gather chunks in phase B
PREP_MODE = os.environ.get("GCN_PREP", "0") == "1"  # prepare_only gathers


def _chunk_tiles(nt, n):
    out, s = [], 0
    for i in range(1, n + 1):
        e = min(max(int(round(i * nt / n)), s + 1), nt)
        out.append((s, e))
        s = e
        if s >= nt:
            break
    return out


def _chunk_tiles_by_blocks(weights, n):
    """Split tiles [0, len(weights)) into <= n contiguous chunks with
    near-equal total weight (block count), preserving order."""
    nt = len(weights)
    total = sum(weights)
    out, s, acc, done = [], 0, 0, 0
    for j in range(nt):
        acc += weights[j]
        remaining_chunks = n - len(out)
        target = (total - done) / remaining_chunks if remaining_chunks else 1
        if acc >= target and j + 1 - s >= 1 and (nt - j - 1) >= (
                remaining_chunks - 1):
            out.append((s, j + 1))
            done += acc
            s, acc = j + 1, 0
        if len(out) == n - 1 and s < nt:
            out.append((s, nt))
            return out
    if s < nt:
        out.append((s, nt))
    return out


def _build_nc(BH):
    """Build the SPMD Bass program.

    BH: [nt][2] list - number of 128-lane blocks per (dst tile, src half),
    identical across cores."""
    f32 = mybir.dt.float32
    bf16 = mybir.dt.bfloat16
    i16 = mybir.dt.int16
    copyf = mybir.ActivationFunctionType.Copy
    relu = mybir.ActivationFunctionType.Relu

    # block offsets: all half-0 groups then all half-1 groups
    boff = [[0, 0] for _ in range(NT)]
    off = 0
    for h in range(2):
        for j in range(NT):
            boff[j][h] = off
            off += BH[j][h]
    totblk = off

    nc = bacc.Bacc("TRN2", num_devices=N_CORES, num_swdge_queues=NQ)

    # ---- kernel I/O ----
    xt = nc.dram_tensor("xt", [D0, SHARD], bf16, kind="ExternalInput")
    w1 = nc.dram_tensor("w1", [D0, DL[0]], bf16, kind="ExternalInput")
    w2 = nc.dram_tensor("w2", [DL[0], DL[1]], bf16, kind="ExternalInput")
    w3 = nc.dram_tensor("w3", [DL[1], DL[2]], bf16, kind="ExternalInput")
    ident = nc.dram_tensor("ident", [T, T], bf16, kind="ExternalInput")
    brows = nc.dram_tensor("brows", [1, sum(DL)], bf16, kind="ExternalInput")
    idx = nc.dram_tensor("idx", [T, totblk * 8], i16, kind="ExternalInput")
    sel = nc.dram_tensor("sel", [T, totblk * T], bf16, kind="ExternalInput")
    out = nc.dram_tensor("out", [SHARD, D3_REAL], f32, kind="ExternalOutput")

    # warmup collective: absorbs SPMD launch skew + ncfw startup so the
    # first real AllGather isn't ~60us late
    wuin = nc.dram_tensor("wuin", [1, 128], bf16)
    wuout = nc.dram_tensor("wuout", [N_CORES, 128], bf16, addr_space="Shared")

    # ---- internal DRAM bounce buffers for the collectives (per layer/half)
    hsz = [HALF_A, HB_ROWS]
    agin = [[nc.dram_tensor(f"agin{l}_{h}", [hsz[h], DL[l]], bf16)
             for h in range(2)] for l in range(3)]
    agout = [[nc.dram_tensor(f"agout{l}_{h}", [N_CORES * hsz[h], DL[l]], bf16,
                             addr_space="Shared")
              for h in range(2)] for l in range(3)]
    rg = [list(range(N_CORES))]

    w_dram = [w1, w2, w3]
    w_chunks = [D0 // T, DL[0] // T, DL[1] // T]

    chunks_a = _chunk_tiles(NT, NCH_A)
    chunks_b = _chunk_tiles(NT, NCH_B)

    with TileContext(nc) as tc:
        nc.gpsimd.load_library(_mlp_lib)

        with (
            tc.tile_pool(name="const", bufs=1) as cpool,
            tc.tile_pool(name="gA", bufs=4) as gpool_a,
            tc.tile_pool(name="gB", bufs=6) as gpool_b,
            tc.tile_pool(name="sA", bufs=4) as spool_a,
            tc.tile_pool(name="sB", bufs=6) as spool_b,
            tc.tile_pool(name="spillp", bufs=1) as spillpool,
            tc.tile_pool(name="hp", bufs=3) as hpool,
            tc.tile_pool(name="zbp", bufs=3) as zbpool,
            tc.tile_pool(name="obp", bufs=3) as obpool,
            tc.tile_pool(name="pxp", bufs=1) as pxpool,
            tc.tile_pool(name="ps_agg", bufs=3, space="PSUM") as ps_agg,
            tc.tile_pool(name="ps_z", bufs=2, space="PSUM") as ps_z,
        ):
            # ---- load constants ----
            def load_const(dram_h, shape, dtype):
                t = cpool.tile(shape, dtype, tag=f"c_{dram_h.name}")
                nc.sync.dma_start(out=t[:, :], in_=dram_h.ap())
                return t

            def load_const_chunked(dram_h, inner, dtype):
                cs = dram_h.shape[0] // T
                t = cpool.tile([T, cs * inner], dtype, tag=f"c_{dram_h.name}")
                nc.sync.dma_start(
                    out=t.rearrange("p (c n) -> p c n", c=cs),
                    in_=dram_h.ap().rearrange("(c p) n -> p c n", p=T),
                )
                return t

            idx_sb = load_const(idx, [T, totblk * 8], i16)
            xt_sb = load_const_chunked(xt, SHARD, bf16)
            w_sb = [load_const_chunked(w_dram[l], DL[l], bf16)
                    for l in range(3)]
            ident_sb = load_const(ident, [T, T], bf16)
            brows_sb = load_const(brows, [1, sum(DL)], bf16)

            out_v = out.ap().rearrange("(n p) d -> p n d", p=T)
            agin_v = [[agin[l][h].ap().rearrange("(n p) d -> p n d", p=T)
                       for h in range(2)] for l in range(3)]

            ag_insts = [[None, None] for _ in range(3)]
            agin_dmas = [[[], []] for _ in range(3)]

            dma_sems = [nc.alloc_semaphore(f"gdma{q}") for q in range(NQ)]
            prep_sems = [nc.alloc_semaphore(f"prep{q}") for q in range(NQ)]
            ag_sems = [[nc.alloc_semaphore(f"agdone{l}_{h}") for h in range(2)]
                       for l in range(3)]
            # cumulative preps emitted per queue, and per-(l, h, q) counts
            prep_cum = [0] * NQ
            fire_info = {}

            # bias rows: one tiny DMA per layer into the half-b buffer
            boff_b = 0
            for l in range(3):
                d = nc.sync.dma_start(
                    out=agin_v[l][1][0:1, NB1, :],
                    in_=brows_sb[0:1, boff_b:boff_b + DL[l]],
                )
                agin_dmas[l][1].append(d)
                boff_b += DL[l]

            def issue_ag(l, h):
                cc = nc.gpsimd.collective_compute(
                    "AllGather",
                    mybir.AluOpType.bypass,
                    replica_groups=rg,
                    ins=[agin[l][h].ap().opt()],
                    outs=[agout[l][h].ap().opt()],
                )
                for d in agin_dmas[l][h]:
                    add_dep_helper(cc.ins, d.ins, reason=f"ag{l}.{h} after dmas")
                ag_insts[l][h] = cc

            def store_z(l, j, zp):
                zb = zbpool.tile([T, DL[l]], bf16, tag="zb")
                nc.vector.tensor_copy(zb[:, :], zp)
                if j < HT_A:
                    d = nc.sync.dma_start(out=agin_v[l][0][:, j, :],
                                          in_=zb[:, :])
                    agin_dmas[l][0].append(d)
                else:
                    d = nc.sync.dma_start(out=agin_v[l][1][:, j - HT_A, :],
                                          in_=zb[:, :])
                    agin_dmas[l][1].append(d)
                if j == HT_A - 1:
                    issue_ag(l, 0)
                if j == NT - 1:
                    issue_ag(l, 1)

            def prep_one(l, h, k, chunks, gpool, spool):
                """Emit one chunk's prepare_only gather + sel load."""
                d_el = DL[l]
                q = k % NQ
                j0, j1 = chunks[k]
                b0 = boff[j0][h]
                b1 = boff[j1 - 1][h] + BH[j1 - 1][h]
                nb = b1 - b0
                gt = gpool.tile([T, nb * d_el], bf16, tag=f"gath{h}",
                                name="gt")
                gt3 = gt.rearrange("p (n d) -> p n d", d=d_el)
                if PREP_MODE:
                    nc.gpsimd.dma_gather(
                        gt3,
                        agout[l][h].ap(),
                        idx_sb[:, b0 * 8:b1 * 8],
                        nb * T,
                        nb * T,
                        d_el,
                        single_packet=False,
                        prepare_only=True,
                        sem=dma_sems[q],
                        queue_num=q,
                    ).then_inc(prep_sems[q], 1)
                    prep_cum[q] += 1
                else:
                    g = nc.gpsimd.dma_gather(
                        gt3,
                        agout[l][h].ap(),
                        idx_sb[:, b0 * 8:b1 * 8],
                        nb * T,
                        nb * T,
                        d_el,
                        single_packet=False,
                        queue_num=q,
                    )
                    add_dep_helper(g.ins, ag_insts[l][h].ins,
                                   reason=f"gath{l}.{h}.{k} after ag")
                st = spool.tile([T, nb * T], bf16, tag=f"sel{h}", name="st")
                st3 = st.rearrange("p (n d) -> p n d", d=T)
                nc.sync.dma_start(out=st[:, :], in_=sel[:, b0 * T:b1 * T])
                return (j0, j1, gt3, st3, b0, q, prep_cum[q])

            def prep_chunks(l, h, chunks, gpool, spool):
                return [prep_one(l, h, k, chunks, gpool, spool)
                        for k in range(len(chunks))]

            def ag_echo(l, h):
                """Bump ag_sems[l][h] when AG(l, h) completes.  The trigger
                instruction's dep resolver drops manual edges and the
                collective can't carry extra sem updates, so a cheap vector
                memset echoes the completion onto a plain semaphore the
                GpSimd stream can wait on."""
                et = pxpool.tile([T, 2], bf16, tag=f"px{l}{h}", name="px")
                em = nc.vector.tensor_copy(et[:, :], ident_sb[:, 0:2])
                add_dep_helper(em.ins, ag_insts[l][h].ins,
                               reason=f"echo{l}.{h} after ag")
                em.then_inc(ag_sems[l][h], 1)

            def chunk_trigger(info, first_lh=None):
                """Fire one chunk's prepared gather.  The first chunk of a
                half waits for that half's AllGather; later chunks inherit
                the gate through the GpSimd FIFO."""
                _, _, _, _, _, q, cum = info
                if first_lh is not None:
                    fl, fh = first_lh
                    ag_echo(fl, fh)
                    nc.gpsimd.wait_ge(ag_sems[fl][fh], 1)
                nc.gpsimd.wait_ge(prep_sems[q], cum)
                nc.gpsimd.trigger_dma(count=1, queue_num=q)

            NA_CH = len(chunks_a)
            NB_CH = len(chunks_b)
            A_BUFS = 6
            B_BUFS = 4
            a_infos = [[None] * NA_CH for _ in range(3)]
            b_infos = [[None] * NB_CH for _ in range(3)]

            # ---- earliest gather preps: up to the pool depth per phase.
            # Emitted before the z loop (and so before any AllGather
            # instruction exists) so descriptor generation on the Q7 DSP
            # pairs free-runs from t=0; the triggers gate the actual DMAs
            # on AllGather completion.
            if PREP_MODE:
                for k in range(A_BUFS):
                    a_infos[0][k] = prep_one(0, 0, k, chunks_a,
                                             gpool_a, spool_a)
                for k in range(B_BUFS):
                    b_infos[0][k] = prep_one(0, 1, k, chunks_b,
                                             gpool_b, spool_b)

            # ---- layer 1 local Z = x @ W1 ----
            for j in range(NT):
                zp = ps_z.tile([T, DL[0]], f32, tag="zpsum")
                for c in range(w_chunks[0]):
                    nc.tensor.matmul(
                        zp[:, :],
                        xt_sb[:, c * SHARD + j * T: c * SHARD + (j + 1) * T],
                        w_sb[0][:, c * DL[0]:(c + 1) * DL[0]],
                        start=(c == 0),
                        stop=(c == w_chunks[0] - 1),
                    )
                store_z(0, j, zp[:, :])

            # ---- aggregation layers ----
            for l in range(3):
                d_el = DL[l]
                cs = d_el // T
                last = l == 2
                if not PREP_MODE and l == 0:
                    a_infos[0] = prep_chunks(0, 0, chunks_a,
                                             gpool_a, spool_a)
                if not PREP_MODE:
                    b_infos[l] = prep_chunks(l, 1, chunks_b,
                                             gpool_b, spool_b)
                spill = spillpool.tile([T, NT * DL[0]], bf16, tag="spill",
                                       name="spill")

                # phase A: half-0 blocks -> psum -> bf16 spill
                for ci in range(NA_CH):
                    (j0, j1, gt3, st3, b0, _, _) = a_infos[l][ci]
                    if PREP_MODE:
                        chunk_trigger(a_infos[l][ci],
                                      first_lh=(l, 0) if ci == 0 else None)
                    for j in range(j0, j1):
                        nb_j = BH[j][0]
                        jb = boff[j][0] - b0
                        pss = [ps_agg.tile([T, T], f32, tag=f"agg{c}",
                                           name=f"ps_agg{c}")
                               for c in range(cs)]
                        for b in range(nb_j):
                            st_b = st3[:, jb + b, :]
                            if last:
                                nc.tensor.matmul(
                                    pss[0][:, :], st_b, gt3[:, jb + b, :],
                                    start=(b == 0), stop=(b == nb_j - 1))
                            else:
                                for c in range(cs):
                                    nc.tensor.matmul(
                                        pss[c][:, :],
                                        gt3[:, jb + b, c * T:(c + 1) * T],
                                        st_b,
                                        start=(b == 0), stop=(b == nb_j - 1))
                        for c in range(cs):
                            nc.scalar.activation(
                                spill[:, j * d_el + c * T:
                                      j * d_el + (c + 1) * T],
                                pss[c][:, :], copyf)
                    # stream this layer's remaining phase-A preps into the
                    # buffer slots just freed
                    if PREP_MODE and ci + A_BUFS < NA_CH:
                        a_infos[l][ci + A_BUFS] = prep_one(
                            l, 0, ci + A_BUFS, chunks_a, gpool_a, spool_a)

                # next layer's phase-A preps (first A_BUFS chunks; the rest
                # stream inside its own phase-A loop).  The reused slots'
                # readers are this layer's phase-A matmuls, already emitted,
                # and CC(l+1, 0) is not issued yet, so descriptor gen
                # overlaps this layer's phase B.
                if PREP_MODE and not last:
                    for k in range(A_BUFS):
                        a_infos[l + 1][k] = prep_one(
                            l + 1, 0, k, chunks_a, gpool_a, spool_a)
                if PREP_MODE and l >= 1:
                    # the phase-B prep deferred past the previous layer's
                    # half-b AllGather issue: its auto RAW on that collective
                    # would stall desc-gen, so it sits here where the first
                    # phase-B trigger waits on the same collective anyway.
                    b_infos[l][B_BUFS - 1] = prep_one(
                        l, 1, B_BUFS - 1, chunks_b, gpool_b, spool_b)

                # phase B: re-inject spill, half-1 blocks (+ bias lane),
                # then post-ops straight out of PSUM
                for ci, (j0, j1, gt3, st3, b0, _, _) in \
                        enumerate(b_infos[l]):
                    if PREP_MODE:
                        chunk_trigger(b_infos[l][ci],
                                      first_lh=(l, 1) if ci == 0 else None)
                    for j in range(j0, j1):
                        nb_j = BH[j][1]
                        jb = boff[j][1] - b0
                        pss = [ps_agg.tile([T, T], f32, tag=f"agg{c}",
                                           name=f"ps_agg{c}")
                               for c in range(cs)]
                        for c in range(cs):
                            nc.tensor.matmul(
                                pss[c][:, :], ident_sb[:, :],
                                spill[:, j * d_el + c * T:
                                      j * d_el + (c + 1) * T],
                                start=True, stop=False)
                        for b in range(nb_j):
                            st_b = st3[:, jb + b, :]
                            if last:
                                nc.tensor.matmul(
                                    pss[0][:, :], st_b, gt3[:, jb + b, :],
                                    start=False, stop=(b == nb_j - 1))
                            else:
                                for c in range(cs):
                                    nc.tensor.matmul(
                                        pss[c][:, :],
                                        gt3[:, jb + b, c * T:(c + 1) * T],
                                        st_b,
                                        start=False, stop=(b == nb_j - 1))
                        if last:
                            ob = obpool.tile([T, D3_REAL], f32, tag="ob")
                            nc.vector.tensor_copy(ob[:, :],
                                                  pss[0][:, :D3_REAL])
                            nc.sync.dma_start(out=out_v[:, j, :],
                                              in_=ob[:, :])
                        else:
                            h_t = hpool.tile([T, d_el], bf16, tag="h")
                            for c in range(cs):
                                nc.scalar.activation(
                                    h_t[:, c * T:(c + 1) * T],
                                    pss[c][:, :], relu)
                            ln = l + 1
                            zp = ps_z.tile([T, DL[ln]], f32, tag="zpsum")
                            for c in range(cs):
                                nc.tensor.matmul(
                                    zp[:, :],
                                    h_t[:, c * T:(c + 1) * T],
                                    w_sb[ln][:, c * DL[ln]:(c + 1) * DL[ln]],
                                    start=(c == 0),
                                    stop=(c == cs - 1),
                                )
                            store_z(ln, j, zp[:, :])
                    # stream phase-B preps: refill the buffer slot this
                    # chunk just freed.  Same-layer chunks first; then the
                    # next layer's first chunks (stopping one short so no
                    # prep lands after the next layer's half-b AllGather is
                    # issued, which would stall desc-gen on its completion).
                    if PREP_MODE:
                        nk = ci + B_BUFS
                        if nk < NB_CH:
                            b_infos[l][nk] = prep_one(
                                l, 1, nk, chunks_b, gpool_b, spool_b)
                        elif not last and nk - NB_CH < B_BUFS - 1:
                            b_infos[l + 1][nk - NB_CH] = prep_one(
                                l + 1, 1, nk - NB_CH, chunks_b,
                                gpool_b, spool_b)
                if not last and not PREP_MODE:
                    a_infos[l + 1] = prep_chunks(l + 1, 0, chunks_a,
                                                 gpool_a, spool_a)

    nc.compile()
    return nc


# ----------------------------------------------------------------------------
# Host-side preprocessing (index work + sharding)
# ----------------------------------------------------------------------------
def _balanced_node_order(deg, n_nodes, nt):
    """Assign nodes to (core, tile) buckets so per-bucket in-edge counts are
    near-equal: sort by degree desc, deal round-robin (snake) over buckets.
    Returns node_order[n_slots] (original node id per slot, -1 for pad) and
    new_pos[n_nodes] (slot of each node)."""
    n_buckets = N_CORES * nt
    slots_total = n_buckets * T
    by_deg = np.argsort(-deg, kind="stable")
    node_order = -np.ones(slots_total, np.int64)
    new_pos = np.zeros(n_nodes, np.int64)
    fill = np.zeros(n_buckets, np.int64)
    b = 0
    direction = 1
    for node in by_deg:
        node_order[b * T + fill[b]] = node
        new_pos[node] = b * T + fill[b]
        fill[b] += 1
        b += direction
        if b == n_buckets:
            b = n_buckets - 1
            direction = -1
        elif b < 0:
            b = 0
            direction = 1
    return node_order, new_pos


def _preprocess(edge_index, n_nodes=N_NODES, nt=NT):
    """Group (self-loop-augmented) edges by (dst tile, src half) per core;
    pad each group to a block multiple of 128 (+1 bias lane per tile in
    half b), block counts maxed across cores.  Selector values carry the
    full symmetric GCN normalization dis[src]*dis[dst]."""
    shard = nt * T
    src = np.asarray(edge_index[0], dtype=np.int64)
    dst = np.asarray(edge_index[1], dtype=np.int64)
    loop = np.arange(n_nodes, dtype=np.int64)
    src = np.concatenate([src, loop])
    dst = np.concatenate([dst, loop])

    deg = np.bincount(dst, minlength=n_nodes).astype(np.float64)
    dis_full = np.where(deg > 0, 1.0 / np.sqrt(deg), 0.0)

    node_order, new_pos = _balanced_node_order(deg, n_nodes, nt)

    dpos = new_pos[dst]
    spos = new_pos[src]
    core_of = dpos // shard
    tile_of = (dpos % shard) // T
    slot_of = dpos % T
    sl = spos % shard
    half_of = (sl >= HALF_A).astype(np.int64)  # 0 or 1
    # row index within the half's gathered buffer (half-b rows incl pad tile)
    stride_h = np.array([HALF_A, HB_ROWS])
    base_h = np.array([0, HALF_A])
    row_of = (spos // shard) * stride_h[half_of] + sl - base_h[half_of]

    val = (dis_full[src] * dis_full[dst]).astype(np.float32)

    counts = np.zeros((N_CORES, nt, 2), np.int64)
    np.add.at(counts, (core_of, tile_of, half_of), 1)
    maxc = counts.max(axis=0)  # [nt, 2]
    need = maxc.copy()
    need[:, 1] += 1  # bias lane per tile in half b
    bh = np.maximum(1, np.ceil(need / T).astype(np.int64))  # [nt, 2]
    BH = bh.tolist()

    # block offsets (half-major), same as the builder
    boff = np.zeros((nt, 2), np.int64)
    off = 0
    for h in range(2):
        for j in range(nt):
            boff[j][h] = off
            off += bh[j][h]
    totblk = int(off)

    order = np.lexsort((tile_of, half_of, core_of))
    row_s = row_of[order]
    core_s = core_of[order]
    tile_s = tile_of[order]
    slot_s = slot_of[order]
    half_s = half_of[order]
    val_s = val[order]

    grp = (core_s * 2 + half_s) * nt + tile_s
    grp_start = np.zeros(N_CORES * 2 * nt + 1, np.int64)
    np.add.at(grp_start, grp + 1, 1)
    grp_start = np.cumsum(grp_start)
    rank = np.arange(len(grp)) - grp_start[grp]

    pos = boff[tile_s, half_s] * T + rank  # padded position within the core
    blk = pos // T
    lane = pos % T

    idx_cores, sel_cores = [], []
    KC = totblk * T
    for c in range(N_CORES):
        m = core_s == c
        idx_pad = np.zeros(KC, np.int16)
        idx_pad[pos[m]] = row_s[m].astype(np.int16)
        selc = np.zeros((totblk, T, T), np.float32)
        selc[blk[m], lane[m], slot_s[m]] = val_s[m]
        # bias lane per tile: first free lane of the tile's half-b group
        for j in range(nt):
            bp = boff[j, 1] * T + counts[c, j, 1]
            idx_pad[bp] = np.int16(c * HB_ROWS + NB1 * T)
            selc[bp // T, bp % T, :] = 1.0
        idx_wrapped = np.tile(
            idx_pad.reshape(KC // 16, 16).T, (8, 1)).astype(np.int16)
        idx_cores.append(np.ascontiguousarray(idx_wrapped))
        sel_cores.append(
            np.ascontiguousarray(
                selc.transpose(1, 0, 2).reshape(T, totblk * T)).astype(BF16))

    return idx_cores, sel_cores, BH, node_order


def _make_in_maps(x, W1, b1, W2, b2, W3, b3, edge_index,
                  n_nodes=N_NODES, nt=NT):
    shard = nt * T
    idx_cores, sel_cores, BH, node_order = _preprocess(
        edge_index, n_nodes, nt)

    x = np.asarray(x, np.float32)
    W3p = np.zeros((DL[1], DL[2]), np.float32)
    W3p[:, :D3_REAL] = np.asarray(W3, np.float32)
    b3p = np.zeros(DL[2], np.float32)
    b3p[:D3_REAL] = np.asarray(b3, np.float32)

    w1b = np.asarray(W1, np.float32).astype(BF16)
    w2b = np.asarray(W2, np.float32).astype(BF16)
    w3b = W3p.astype(BF16)
    brows = np.concatenate([
        np.asarray(b1, np.float32),
        np.asarray(b2, np.float32),
        b3p,
    ]).reshape(1, -1).astype(BF16)
    identity = np.eye(T, dtype=BF16)

    in_maps = []
    for c in range(N_CORES):
        slots = node_order[c * shard:(c + 1) * shard]
        xs = np.where((slots >= 0)[:, None], x[np.maximum(slots, 0)], 0.0)
        xs = xs.astype(np.float32)
        in_maps.append({
            "xt": np.ascontiguousarray(xs.T).astype(BF16),
            "w1": w1b, "w2": w2b, "w3": w3b,
            "ident": identity,
            "brows": brows,
            "idx": idx_cores[c],
            "sel": sel_cores[c],
        })
    return in_maps, BH, node_order


_NC_CACHE = {}


def kernel_with_results(x, W1, b1, W2, b2, W3, b3, edge_index, trace=False):
    in_maps, BH, node_order = _make_in_maps(
        x, W1, b1, W2, b2, W3, b3, edge_index)
    key = tuple(tuple(r) for r in BH)
    if key not in _NC_CACHE:
        _NC_CACHE[key] = _build_nc(BH)
    nc = _NC_CACHE[key]
    res = run_bass_kernel_spmd(
        nc, in_maps, core_ids=list(range(N_CORES)), trace=trace
    )
    rows = np.concatenate(
        [np.asarray(res.results[c]["out"]) for c in range(N_CORES)], axis=0)
    full = np.zeros((N_NODES, rows.shape[1]), np.float32)
    real = node_order >= 0
    full[node_order[real]] = rows[real]
    return full, res


def kernel(x, W1, b1, W2, b2, W3, b3, edge_index):
    full, _ = kernel_with_results(x, W1, b1, W2, b2, W3, b3, edge_index)
    return full
